# revision 34
# baseline (speedup 1.0000x reference)
"""TRN2 Bass kernel for BasicLSTM (B=32, T=512, IN=512, H=1024).

Strategy: tensor-parallel over the 4H gate dim across 8 cores.
  - Core k owns gate columns [i_k | f_k | o_k | g_k], each a 128-wide
    H-slice (H indices k*128:(k+1)*128), i.e. 512 gate cols per core.
  - Phase A: xzb = x @ W_k + b_k for all (t, b) rows, stored t-major in
    DRAM scratch ([T*B, 512]).  One big GEMM, near-roofline.
  - Phase B: 512 sequential steps.  Each step:
      z   = hT.T @ U_k + xzb_t      (8 K-chunk matmuls + identity-matmul)
      i,f,o = sigmoid(z[:, :384]); g = tanh(z[:, 384:])
      c   = f*c + i*g;  h = o * tanh(c)         ([32, 128] per core)
      h^T shard -> shared-DRAM psh[pid*128:...] (plain DMA, runtime offset)
      remote_sem_update_broadcast to all 8 cores (+2 arrival credit each)
      wait arrv_sem >= 16*step, then gather full h^T from psh -> SBUF.
    This replaces the per-step AllGather collective (~11us/call fixed
    cost, the old bottleneck) with a ~1us sem-only SWDGE broadcast.
    Safety: cumulative arrival counting (sends(any core) <= min(sends)+1
    by the data-dependency lockstep, so count >= 16*t implies every peer
    wrote step t-1), double-buffered psh, and a clear-sems + AllGather
    barrier prologue so re-executions of the NEFF start clean.
  - Output: core k writes hs[:, :, k*128:(k+1)*128]; host concatenates.
"""

import numpy as np

import concourse.bass as bass
import concourse.mybir as mybir
import concourse.tile as tile
from concourse import bacc, bass_utils
from concourse.bass import ts, ds
from concourse.masks import make_identity
from bass_rust import add_dep_helper

B = 32
T = 512
IN = 512
H = 1024
NCORES = 8
NS = 4 * H // NCORES  # 512 gate cols per core
HS = H // NCORES      # 128 h cols per core
F32 = mybir.dt.float32
F32R = mybir.dt.float32r
BF16 = mybir.dt.bfloat16
AF = mybir.ActivationFunctionType
EXCHANGE = "rdma"  # "rdma" (sem-broadcast barrier) | "ag" (old AllGather)
WARM = 14  # dummy PE transposes per step to hold the clock at full pstate


def _build(t_steps: int = T, ablate: frozenset = frozenset(), reps: int = 1):
    """ablate (perf experiments only, breaks numerics):
    'aonly' - phase A only;  'noex' - skip sem broadcast + wait (stale hT);
    'nomm'  - skip the 8 recurrent matmuls.
    reps > 1 repeats the whole workload in-program (for slope timing)."""
    assert t_steps % 4 == 0
    nc = bacc.Bacc("TRN2", debug=False, num_devices=NCORES)

    x_d = nc.dram_tensor("x", [B, t_steps, IN], F32, kind="ExternalInput")
    w_d = nc.dram_tensor("w", [IN, NS], F32, kind="ExternalInput")
    u_d = nc.dram_tensor("u", [H, NS], F32, kind="ExternalInput")
    b_d = nc.dram_tensor("b", [1, NS], F32, kind="ExternalInput")
    hs_d = nc.dram_tensor("hs", [B, t_steps, HS], F32, kind="ExternalOutput")
    xzb_d = nc.dram_tensor("xzb", [t_steps * B, NS], F32R)
    RG = [list(range(NCORES))]
    bar_in = nc.dram_tensor("bar_in", [1, B], F32R)
    bar_out = nc.dram_tensor("bar_out", [NCORES, B], F32R)
    cc_in = [nc.dram_tensor(f"cc_in{i}", [HS, B], F32R) for i in range(2)]
    cc_out = [
        nc.dram_tensor(f"cc_out{i}", [H, B], F32R, addr_space="Shared")
        for i in range(2)
    ]

    # Long-lived constants/weights: static SBUF allocations (outside tile
    # pools, so the slot allocator can never alias them with rotating tiles).
    id128 = nc.alloc_sbuf_tensor("id128", [128, 128], F32).ap()
    id32 = nc.alloc_sbuf_tensor("id32", [32, 32], F32R).ap()
    ones1 = nc.alloc_sbuf_tensor("ones1", [1, 128], F32R).ap()
    b_sb = nc.alloc_sbuf_tensor("b_sb", [1, NS], F32R).ap()
    wk = [nc.alloc_sbuf_tensor(f"wk{j}", [128, NS], F32R).ap() for j in range(IN // 128)]
    uk = [nc.alloc_sbuf_tensor(f"uk{j}", [128, NS], F32R).ap() for j in range(H // 128)]
    c_bufs = [nc.alloc_sbuf_tensor(f"c_st{i}", [B, HS], F32).ap() for i in range(2)]
    # Direct-receive h^T buffers: remote cores rdma-broadcast their 32-col
    # shard into [:, pid*32:(pid+1)*32]; last 4 cols are a local-write pad
    # that threads the Tile dependency through the critical section.
    hT_recv = [
        nc.alloc_sbuf_tensor(f"hTr{i}", [128, H // 128 * B + 4], F32R).ap()
        for i in range(2)
    ]

    with tile.TileContext(nc) as tc:
        with (
            tc.tile_pool(name="xin", bufs=3) as xin_pool,
            tc.tile_pool(name="xtr", bufs=4) as xt_pool,
            tc.tile_pool(name="xzsb", bufs=3) as xzsb_pool,
            tc.tile_pool(name="psA", bufs=2, space=bass.MemorySpace.PSUM) as psA_pool,
            tc.tile_pool(name="psT", bufs=2, space=bass.MemorySpace.PSUM) as psT_pool,
            tc.tile_pool(name="xzt", bufs=6) as xzt_pool,
            tc.tile_pool(name="state", bufs=2) as st_pool,
            tc.tile_pool(name="gates", bufs=2) as g_pool,
            tc.tile_pool(name="hx", bufs=2) as hx_pool,
            tc.tile_pool(name="psX", bufs=2, space=bass.MemorySpace.PSUM) as psX_pool,
            tc.tile_pool(name="hT", bufs=2) as hT_pool,
            tc.tile_pool(name="psB", bufs=1, space=bass.MemorySpace.PSUM) as psB_pool,
        ):
            nc.any.memset(c_bufs[0], 0.0)
            make_identity(nc, id128)
            id32_dram = nc.inline_tensor(np.eye(32, dtype=np.float32), name="id32c")
            nc.gpsimd.dma_start(id32, id32_dram.ap())
            ones_dram = nc.inline_tensor(np.ones((1, 128), np.float32), name="ones1c")
            nc.gpsimd.dma_start(ones1, ones_dram.ap())
            nc.gpsimd.dma_start(b_sb, b_d.ap())
            for j in range(IN // 128):
                nc.gpsimd.dma_start(wk[j], w_d.ap()[ts(j, 128), :])
            for j in range(H // 128):
                nc.gpsimd.dma_start(uk[j], u_d.ap()[ts(j, 128), :])

            if EXCHANGE == "rdma":
                # Prologue: clear the cross-core sems, then barrier so no
                # core's step-0 credit can arrive before a peer's clear.
                arrv_sem = nc.alloc_semaphore("arrv_sem")
                lsem = nc.alloc_semaphore("rdl_sem")
                cl0 = nc.gpsimd.sem_clear(arrv_sem)
                cl1 = nc.gpsimd.sem_clear(lsem)
                barz = nc.inline_tensor(
                    np.zeros((1, B), np.float32), name="barz"
                )
                bz = nc.gpsimd.dma_start(
                    bar_in.ap().bitcast(F32), barz.ap()
                )
                bar = nc.gpsimd.collective_compute(
                    "AllGather",
                    mybir.AluOpType.bypass,
                    replica_groups=RG,
                    ins=[bar_in.ap().opt()],
                    outs=[bar_out.ap().opt()],
                )
                add_dep_helper(bar.ins, cl0.ins, reason="barrier after sem clear")
                add_dep_helper(bar.ins, cl1.ins, reason="barrier after sem clear")
                add_dep_helper(bar.ins, bz.ins, reason="barrier after input init")
                pad_d = nc.inline_tensor(
                    np.zeros((128, 1), np.float32), name="padz"
                )
                pid = nc.gpsimd.partition_id()
                pofs = pid * B  # my 32-col slot in the receive buffers
                prev_gp = bar  # chains prep/trigger FIFO order on gpsimd
                g_cnt = 0      # global exchange counter (across reps)

            for _rep in range(reps):
                # Phase A: xzb[t*B + b, :] = x[b, t, :] @ W_k + b_k  (t-major rows)
                ntiles = t_steps * B // 128
                for m in range(ntiles):
                    t0 = m * 4
                    xt_in = xin_pool.tile([128, IN], F32, tag="xin")
                    nc.sync.dma_start(
                        xt_in[:, :],
                        x_d.ap()[:, ds(t0, 4), :].rearrange("b t i -> t b i"),
                    )
                    zp = psA_pool.tile([128, NS], F32, tag="zpa")
                    for j in range(IN // 128):
                        xTp = psT_pool.tile([128, 128], F32, tag="xTp")
                        nc.tensor.transpose(xTp, xt_in[:, ts(j, 128)], id128)
                        xTs = xt_pool.tile([128, 128], F32R, tag="xTs")
                        nc.vector.tensor_copy(xTs, xTp)
                        nc.tensor.matmul(
                            zp, xTs, wk[j],
                            start=(j == 0), stop=False,
                        )
                    nc.tensor.matmul(
                        zp, ones1, b_sb,
                        start=False, stop=True,
                    )
                    xz_sb = xzsb_pool.tile([128, NS], F32R, tag="xzsb")
                    nc.vector.tensor_copy(xz_sb, zp)
                    nc.sync.dma_start(xzb_d.ap()[ts(m, 128), :], xz_sb)

                # Phase B: the recurrence.
                hT_cur = None  # h_0 == 0 -> step 0 skips the recurrent matmuls
                for t in range(t_steps if "aonly" not in ablate else 0):
                    xzt = xzt_pool.tile([B, NS], F32R, tag="xzt")
                    nc.scalar.dma_start(xzt, xzb_d.ap()[ts(t, B), :])

                    if EXCHANGE == "rdma" and t < t_steps - 1:
                        # Pre-generate this step's broadcast descriptors now:
                        # descgen only encodes the htx ADDRESS (slot t%2, last
                        # written at t-2), so it runs during the compute and
                        # stays off the critical path.  The data is read at
                        # trigger time, which we gate on the htx write below.
                        htx = hx_pool.tile([128, B], F32R, tag="htx")
                        if "noex" not in ablate:
                            prep = nc.gpsimd.remote_dma_broadcast(
                                hT_recv[t % 2][:, ds(pofs, B)],
                                htx[:, :],
                                remote_sem=arrv_sem,
                                local_sem=lsem,
                                rdests=[(0, m) for m in range(NCORES)],
                            )
                            add_dep_helper(
                                prep.ins, prev_gp.ins,
                                reason="SWDGE FIFO order: prep after prev trigger",
                            )

                    # Gate columns per core: [i | g | f | o].  Two PSUM
                    # halves so the [i|g] activations + i*g overlap the
                    # [f|o] half's matmuls.
                    zpA = psB_pool.tile([B, 2 * HS], F32, tag="zA")
                    zpB = psB_pool.tile([B, 2 * HS], F32, tag="zB")
                    if t == 0 or "nomm" in ablate:
                        nc.tensor.matmul(zpA, id32, xzt[:, 0:2 * HS],
                                         start=True, stop=True)
                        nc.tensor.matmul(zpB, id32, xzt[:, 2 * HS:NS],
                                         start=True, stop=True)
                    else:
                        # xzb add first: it only needs the prefetched xzt, so
                        # it runs during the previous step's exchange window.
                        nc.tensor.matmul(zpA, id32, xzt[:, 0:2 * HS],
                                         start=True, stop=False)
                        nc.tensor.matmul(zpB, id32, xzt[:, 2 * HS:NS],
                                         start=True, stop=False)
                        for j in range(H // 128):
                            nc.tensor.matmul(
                                zpA, hT_cur[:, ts(j, 32)], uk[j][:, 0:2 * HS],
                                start=False, stop=(j == H // 128 - 1),
                            )
                        for j in range(H // 128):
                            nc.tensor.matmul(
                                zpB, hT_cur[:, ts(j, 32)], uk[j][:, 2 * HS:NS],
                                start=False, stop=(j == H // 128 - 1),
                            )

                    si = g_pool.tile([B, HS], F32, tag="si")
                    nc.scalar.activation(si, zpA[:, 0:HS], AF.Sigmoid)
                    g_t = g_pool.tile([B, HS], F32, tag="g")
                    nc.scalar.activation(g_t, zpA[:, HS:2 * HS], AF.Tanh)
                    ig = g_pool.tile([B, HS], F32, tag="ig")
                    nc.vector.tensor_mul(ig, si, g_t)

                    sf = g_pool.tile([B, HS], F32, tag="sf")
                    nc.scalar.activation(sf, zpB[:, 0:HS], AF.Sigmoid)
                    so = g_pool.tile([B, HS], F32, tag="so")
                    nc.scalar.activation(so, zpB[:, HS:2 * HS], AF.Sigmoid)

                    fc = g_pool.tile([B, HS], F32, tag="fc")
                    nc.vector.tensor_mul(fc, sf, c_bufs[t % 2])
                    c_new = c_bufs[(t + 1) % 2]
                    nc.vector.tensor_add(c_new, ig, fc)

                    tc_t = g_pool.tile([B, HS], F32, tag="tc")
                    nc.scalar.activation(tc_t, c_new, AF.Tanh)
                    h_t = st_pool.tile([B, HS], F32R, tag="h")
                    nc.vector.tensor_mul(h_t, so, tc_t)

                    if t == t_steps - 1:
                        nc.scalar.dma_start(
                            hs_d.ap()[:, ds(t, 1), :], h_t.bitcast(F32)
                        )
                        break

                    buf = t % 2
                    if EXCHANGE == "rdma":
                        # Full transpose h [32,128] -> h^T [128,32] on the
                        # tensor engine (it just went idle), then stage in
                        # SBUF for the rdma broadcast.
                        hTp = psX_pool.tile([128, B], F32R, tag="hTp")
                        nc.tensor.transpose(hTp, h_t, id32)
                        cp = nc.vector.tensor_copy(htx, hTp)
                        for _w in range(WARM):
                            wp = psT_pool.tile([128, 128], F32, tag="xTp")
                            nc.tensor.transpose(wp, id128, id128)
                        g_cnt += 1
                        if "noex" not in ablate:
                            trig = nc.gpsimd.trigger_dma(count=1)
                            add_dep_helper(
                                trig.ins, cp.ins,
                                reason="fire broadcast after h^T staged",
                            )
                            prev_gp = trig
                            # The wait's sem rises only via remote SWDGE
                            # credits, which Tile's no-exec scheduling sim
                            # cannot model — a bare wait deadlocks it. A
                            # critical section is opaque to the scheduler;
                            # its boundary nops carry full-tensor deps for
                            # everything the body touches (the pad write
                            # makes hT_recv one of those tensors).
                            with tc.tile_critical(name=f"hx{t}"):
                                # Reads htx so pre_crit (whose deps are the
                                # body's tensor inputs) waits for the staged
                                # h^T — ordering the transpose/copy/trigger
                                # chain ahead of the critical in the
                                # scheduler's engine streams.  Writes the
                                # hT_recv pad column so post_crit carries
                                # the recv buffer as a produced tensor.
                                nc.vector.tensor_copy(
                                    hT_recv[buf][0:1,
                                                 H // 128 * B:H // 128 * B + 1],
                                    htx[0:1, 0:1],
                                )
                                wt = nc.sync.wait_ge(arrv_sem, 16 * g_cnt)
                                add_dep_helper(
                                    wt.ins, trig.ins,
                                    reason="own trigger before blocking wait",
                                )
                                if g_cnt == 1:
                                    add_dep_helper(
                                        wt.ins, bar.ins,
                                        reason="first wait after barrier prologue",
                                    )
                        # hs output write AFTER the critical: anything
                        # emitted before it would gate the critical's entry
                        # (the gate snapshots every engine's prior work).
                        nc.scalar.dma_start(
                            hs_d.ap()[:, ds(t, 1), :], h_t.bitcast(F32)
                        )
                        hT_cur = hT_recv[buf]
                        continue
                    # --- old AllGather path (EXCHANGE == "ag") ---
                    htr = g_pool.tile([B, HS], F32, tag="htr")
                    nc.vector.transpose(htr, h_t.bitcast(F32))
                    nc.sync.dma_start(
                        cc_in[buf].ap().bitcast(htr.dtype)
                        .rearrange("(j p) q -> p j q", j=4),
                        htr.rearrange("p (j q) -> p j q", j=4),
                    )
                    nc.gpsimd.collective_compute(
                        "AllGather",
                        mybir.AluOpType.bypass,
                        replica_groups=RG,
                        ins=[cc_in[buf].ap().opt()],
                        outs=[cc_out[buf].ap().opt()],
                    )
                    hT_new = hT_pool.tile([128, H // 128 * B], F32R, tag="hT")
                    nc.sync.dma_start(
                        hT_new.rearrange("p (j b) -> p j b", j=H // 128),
                        cc_out[buf].ap().rearrange("(j p) b -> p j b", p=128),
                    )
                    hT_cur = hT_new

    nc.compile()
    return nc


def _make_in_maps(x, W, U, b, t_steps: int = T):
    x = np.asarray(x, np.float32)[:, :t_steps, :]
    W = np.asarray(W, np.float32)
    U = np.asarray(U, np.float32)
    b = np.asarray(b, np.float32)
    in_maps = []
    for k in range(NCORES):
        # per-core gate column order: [i | g | f | o]
        cols = np.concatenate(
            [np.arange(k * HS, (k + 1) * HS) + gofs * H for gofs in (0, 2, 1, 3)]
        )
        in_maps.append(
            {
                "x": np.ascontiguousarray(x),
                "w": np.ascontiguousarray(W[:, cols]),
                "u": np.ascontiguousarray(U[:, cols]),
                "b": np.ascontiguousarray(b[cols]).reshape(1, NS),
            }
        )
    return in_maps


def _pjrt_bundle(nc, n_reps: int = 1):
    """Reusable sharded PJRT executable (mirrors bass2jax.run_bass_via_pjrt's
    multi-core branch, but keeps the jitted callable for repeated runs).

    n_reps > 1 chains that many NEFF executions inside one jit call by
    threading the donated output buffer through, so fixed dispatch overhead
    can be cancelled via slope timing."""
    import jax
    from jax.experimental.shard_map import shard_map
    from jax.sharding import Mesh, PartitionSpec
    from concourse import bass2jax

    bass2jax.install_neuronx_cc_hook()
    partition_name = nc.partition_id_tensor.name if nc.partition_id_tensor else None
    in_names, out_names, out_avals, zero_outs = [], [], [], []
    for alloc in nc.m.functions[0].allocations:
        if not isinstance(alloc, mybir.MemoryLocationSet):
            continue
        name = alloc.memorylocations[0].name
        if alloc.kind == "ExternalInput":
            if name != partition_name:
                in_names.append(name)
        elif alloc.kind == "ExternalOutput":
            shape = tuple(alloc.tensor_shape)
            dtype = mybir.dt.np(alloc.dtype)
            out_names.append(name)
            out_avals.append(jax.core.ShapedArray(shape, dtype))
            zero_outs.append(np.zeros(shape, dtype))
    n_params = len(in_names)
    n_outs = len(out_avals)
    all_in_names = list(in_names) + list(out_names)
    if partition_name is not None:
        all_in_names.append(partition_name)

    def _body(*args):
        ins = list(args[:n_params])
        zs = list(args[n_params:])
        for _ in range(n_reps):
            operands = ins + zs
            if partition_name is not None:
                operands.append(bass2jax.partition_id_tensor())
            outs = bass2jax._bass_exec_p.bind(
                *operands,
                out_avals=tuple(out_avals),
                in_names=tuple(all_in_names),
                out_names=tuple(out_names),
                lowering_input_output_aliases=(),
                sim_require_finite=True,
                sim_require_nnan=True,
                nc=nc,
            )
            zs = list(outs)
        return tuple(outs)

    devices = jax.devices()[:NCORES]
    mesh = Mesh(np.asarray(devices), ("core",))
    in_specs = (PartitionSpec("core"),) * (n_params + n_outs)
    out_specs = (PartitionSpec("core"),) * n_outs
    sharded = jax.jit(
        shard_map(
            _body, mesh=mesh, in_specs=in_specs, out_specs=out_specs, check_rep=False
        ),
        donate_argnums=tuple(range(n_params, n_params + n_outs)),
        keep_unused=True,
    )
    return dict(
        fn=sharded,
        mesh=mesh,
        in_names=in_names,
        out_names=out_names,
        out_avals=out_avals,
        zero_outs=zero_outs,
        n_params=n_params,
    )


def _run(inputs, t_steps: int = T, trace: bool = False):
    nc = _build(t_steps)
    in_maps = _make_in_maps(inputs["x"], inputs["W"], inputs["U"], inputs["b"], t_steps)
    res = bass_utils.run_bass_kernel_spmd(
        nc, in_maps, core_ids=list(range(NCORES)), trace=trace
    )
    out = np.empty((B, t_steps, H), np.float32)
    for k in range(NCORES):
        out[:, :, k * HS:(k + 1) * HS] = res.results[k]["hs"]
    return out, res


def kernel(**inputs) -> np.ndarray:
    out, _ = _run(inputs)
    return out


# revision 40
# speedup vs baseline: 1.0138x; 1.0138x over previous
"""TRN2 Bass kernel for BasicLSTM (B=32, T=512, IN=512, H=1024).

Strategy: tensor-parallel over the 4H gate dim across 8 cores, with the
batch split into TWO independent 16-row chains pipelined half a step
apart so each chain's exchange latency hides behind the other's compute.

  - Core k owns gate columns [i_k | f_k | o_k | g_k], each a 128-wide
    H-slice (H indices k*128:(k+1)*128), i.e. 512 gate cols per core.
  - Phase A: xzb = x @ W_k + b_k for all (t, b) rows, stored t-major in
    DRAM scratch ([T*B, 512]).  One big GEMM, near-roofline.
  - Phase B: 512 sequential steps; per step, per chain c in {0, 1}
    (batch rows 16c:16c+16):
      z   = hT_c.T @ U_k + xzb_t,c     (8 K-chunk matmuls + id-matmul)
      i,f,o = sigmoid(...); g = tanh(...); c,h elementwise  [16, 128]
      h^T  = PE transpose -> SBUF stage htx_c
      remote_dma_broadcast htx_c -> all 8 cores' hT_recv[c][t%2] at
      column pid*16 (register dst offset); each dest gets +2 arrival
      credits on arrv_sem[c] only after the data bytes land.
      Receiver waits arrv_sem[c] >= 16*step inside a tile_critical
      (opaque to Tile's scheduling sim), then the next step's matmuls
      read hT_recv directly.
    Safety: cumulative credit counting is sound because the recurrence
    chain itself bounds skew (sends(any core) <= min(sends)+1); receive
    buffers are double-buffered per chain; a clear-sems + AllGather
    barrier prologue makes NEFF re-executions start clean.
  - Output: core k writes hs[:, :, k*128:(k+1)*128]; host concatenates.
"""

import numpy as np

import concourse.bass as bass
import concourse.mybir as mybir
import concourse.tile as tile
from concourse import bacc, bass_utils
from concourse.bass import ts, ds
from concourse.masks import make_identity
from bass_rust import add_dep_helper

B = 32
BC = 16               # batch rows per chain
NCH = 2               # pipelined chains
T = 512
IN = 512
H = 1024
NCORES = 8
NS = 4 * H // NCORES  # 512 gate cols per core
HS = H // NCORES      # 128 h cols per core
F32 = mybir.dt.float32
F32R = mybir.dt.float32r
AF = mybir.ActivationFunctionType
WARM = 5  # dummy PE transposes per gap (clock-ramp hold); 0 = off


def _build(t_steps: int = T, ablate: frozenset = frozenset(), reps: int = 1):
    """ablate (perf experiments only, breaks numerics):
    'aonly' - phase A only;  'noex' - skip broadcasts + waits (stale hT);
    'nomm'  - skip the recurrent matmuls.
    reps > 1 repeats the whole workload in-program (for slope timing)."""
    assert t_steps % 4 == 0
    nc = bacc.Bacc("TRN2", debug=False, num_devices=NCORES)

    x_d = nc.dram_tensor("x", [B, t_steps, IN], F32, kind="ExternalInput")
    w_d = nc.dram_tensor("w", [IN, NS], F32, kind="ExternalInput")
    u_d = nc.dram_tensor("u", [H, NS], F32, kind="ExternalInput")
    b_d = nc.dram_tensor("b", [1, NS], F32, kind="ExternalInput")
    hs_d = nc.dram_tensor("hs", [B, t_steps, HS], F32, kind="ExternalOutput")
    xzb_d = nc.dram_tensor("xzb", [t_steps * B, NS], F32R)
    RG = [list(range(NCORES))]
    bar_in = nc.dram_tensor("bar_in", [1, B], F32R)
    bar_out = nc.dram_tensor("bar_out", [NCORES, B], F32R)

    # Long-lived constants/weights: static SBUF allocations (outside tile
    # pools, so the slot allocator can never alias them with rotating tiles).
    id128 = nc.alloc_sbuf_tensor("id128", [128, 128], F32).ap()
    id16 = nc.alloc_sbuf_tensor("id16", [BC, BC], F32R).ap()
    ones1 = nc.alloc_sbuf_tensor("ones1", [1, 128], F32R).ap()
    b_sb = nc.alloc_sbuf_tensor("b_sb", [1, NS], F32R).ap()
    wk = [nc.alloc_sbuf_tensor(f"wk{j}", [128, NS], F32R).ap() for j in range(IN // 128)]
    uk = [nc.alloc_sbuf_tensor(f"uk{j}", [128, NS], F32R).ap() for j in range(H // 128)]
    c_st = [
        [nc.alloc_sbuf_tensor(f"c_st{c}_{i}", [BC, HS], F32).ap() for i in range(2)]
        for c in range(NCH)
    ]
    # Direct-receive h^T buffers per chain: remote cores rdma-broadcast
    # their 16-col shard into [:, pid*16:(pid+1)*16]; the last 4 cols are
    # a local-write pad that threads Tile deps through the critical.
    hT_recv = [
        [
            nc.alloc_sbuf_tensor(f"hTr{c}_{i}", [128, NCORES * BC + 4], F32R).ap()
            for i in range(2)
        ]
        for c in range(NCH)
    ]
    PADC = NCORES * BC  # pad column index
    RSTEPS = 32  # h output ring: steps buffered in SBUF before one bulk DMA
    hring = [
        [
            nc.alloc_sbuf_tensor(f"hring{c}_{i}", [BC, RSTEPS * HS], F32R).ap()
            for i in range(2)
        ]
        for c in range(NCH)
    ]

    with tile.TileContext(nc) as tc:
        with (
            tc.tile_pool(name="xin", bufs=3) as xin_pool,
            tc.tile_pool(name="xtr", bufs=4) as xt_pool,
            tc.tile_pool(name="xzsb", bufs=3) as xzsb_pool,
            tc.tile_pool(name="psA", bufs=2, space=bass.MemorySpace.PSUM) as psA_pool,
            tc.tile_pool(name="psT", bufs=2, space=bass.MemorySpace.PSUM) as psT_pool,
            tc.tile_pool(name="xzt", bufs=4) as xzt_pool,
            tc.tile_pool(name="state", bufs=2) as st_pool,
            tc.tile_pool(name="gates", bufs=2) as g_pool,
            tc.tile_pool(name="hx", bufs=2) as hx_pool,
            tc.tile_pool(name="psX", bufs=1, space=bass.MemorySpace.PSUM) as psX_pool,
            tc.tile_pool(name="psB", bufs=1, space=bass.MemorySpace.PSUM) as psB_pool,
        ):
            for c in range(NCH):
                nc.any.memset(c_st[c][0], 0.0)
            make_identity(nc, id128)
            id16_dram = nc.inline_tensor(np.eye(BC, dtype=np.float32), name="id16c")
            nc.gpsimd.dma_start(id16, id16_dram.ap())
            ones_dram = nc.inline_tensor(np.ones((1, 128), np.float32), name="ones1c")
            nc.gpsimd.dma_start(ones1, ones_dram.ap())
            nc.gpsimd.dma_start(b_sb, b_d.ap())
            for j in range(IN // 128):
                nc.gpsimd.dma_start(wk[j], w_d.ap()[ts(j, 128), :])
            for j in range(H // 128):
                nc.gpsimd.dma_start(uk[j], u_d.ap()[ts(j, 128), :])

            # Prologue: clear the cross-core sems, then barrier so no core's
            # step-0 credit can arrive before a peer's clear.
            arrv = [nc.alloc_semaphore(f"arrv{c}") for c in range(NCH)]
            lsem = nc.alloc_semaphore("rdl_sem")
            cls = [nc.gpsimd.sem_clear(s) for s in arrv + [lsem]]
            barz = nc.inline_tensor(np.zeros((1, B), np.float32), name="barz")
            bz = nc.gpsimd.dma_start(bar_in.ap().bitcast(F32), barz.ap())
            bar = nc.gpsimd.collective_compute(
                "AllGather",
                mybir.AluOpType.bypass,
                replica_groups=RG,
                ins=[bar_in.ap().opt()],
                outs=[bar_out.ap().opt()],
            )
            for cl in cls:
                add_dep_helper(bar.ins, cl.ins, reason="barrier after sem clear")
            add_dep_helper(bar.ins, bz.ins, reason="barrier after input init")
            pid = nc.gpsimd.partition_id()
            pofs = pid * BC  # my 16-col slot in the receive buffers
            prev_gp = bar    # chains prep/trigger FIFO order on gpsimd
            g_ex = [0, 0]    # per-chain exchange counters (across reps)

            def emit_phase_a_tile(m):
                # xzb[(4m..4m+4)*B + b, :] = x @ W_k + b_k (one 128-row tile)
                t0 = m * 4
                xt_in = xin_pool.tile([128, IN], F32, tag="xin")
                nc.sync.dma_start(
                    xt_in[:, :],
                    x_d.ap()[:, ds(t0, 4), :].rearrange("b t i -> t b i"),
                )
                zp = psA_pool.tile([128, NS], F32, tag="zpa")
                for j in range(IN // 128):
                    xTp = psT_pool.tile([128, 128], F32, tag="xTp")
                    nc.tensor.transpose(xTp, xt_in[:, ts(j, 128)], id128)
                    xTs = xt_pool.tile([128, 128], F32R, tag="xTs")
                    nc.vector.tensor_copy(xTs, xTp)
                    nc.tensor.matmul(
                        zp, xTs, wk[j],
                        start=(j == 0), stop=False,
                    )
                nc.tensor.matmul(
                    zp, ones1, b_sb,
                    start=False, stop=True,
                )
                xz_sb = xzsb_pool.tile([128, NS], F32R, tag="xzsb")
                nc.vector.tensor_copy(xz_sb, zp)
                nc.sync.dma_start(xzb_d.ap()[ts(m, 128), :], xz_sb)

            for _rep in range(reps):
                # Phase A is interleaved into phase B's exchange-wait gaps
                # (one tile per two steps): real PE work that also keeps the
                # clock ramped.  Two tiles up front give the pipeline margin.
                ntiles = t_steps * B // 128
                for m in range(min(2, ntiles)):
                    emit_phase_a_tile(m)

                # Phase B: software-pipelined two-chain recurrence.
                # Iteration t emits: [chain-1 tail of step t-1] [both
                # matmul blocks of step t] [chain-0 tail of step t], so
                # chain 1's acts/exchange run under chain 0's matmuls and
                # vice versa -- each chain's exchange latency hides behind
                # the other chain's compute.
                hT_cur = [None, None]  # h_0 == 0 -> step 0 skips recur. mm
                zps_pend = [None, None]
                htx_pend = [None, None]

                def flush_ring(tf):
                    n_fl = tf % RSTEPS + 1
                    t0f = tf - n_fl + 1
                    for c in range(NCH):
                        nc.sync.dma_start(
                            hs_d.ap()[ds(BC * c, BC), ds(t0f, n_fl), :],
                            hring[c][(tf // RSTEPS) % 2][
                                :, 0:n_fl * HS
                            ].bitcast(F32).rearrange("b (s h) -> b s h", h=HS),
                        )

                def emit_tail(c, t):
                    nonlocal prev_gp
                    zp = zps_pend[c]
                    sif = g_pool.tile([BC, 3 * HS], F32, tag=f"sif{c}")
                    nc.scalar.activation(sif, zp[:, 0:3 * HS], AF.Sigmoid)
                    g_t = g_pool.tile([BC, HS], F32, tag=f"g{c}")
                    nc.scalar.activation(g_t, zp[:, 3 * HS:4 * HS], AF.Tanh)
                    so = sif[:, 2 * HS:3 * HS]

                    fc = g_pool.tile([BC, HS], F32, tag=f"fc{c}")
                    nc.vector.tensor_mul(fc, sif[:, HS:2 * HS], c_st[c][t % 2])
                    ig = g_pool.tile([BC, HS], F32, tag=f"ig{c}")
                    nc.vector.tensor_mul(ig, sif[:, 0:HS], g_t)
                    c_new = c_st[c][(t + 1) % 2]
                    nc.vector.tensor_add(c_new, ig, fc)

                    tc_t = g_pool.tile([BC, HS], F32, tag=f"tc{c}")
                    nc.scalar.activation(tc_t, c_new, AF.Tanh)
                    h_t = hring[c][(t // RSTEPS) % 2][:, ds((t % RSTEPS) * HS, HS)]
                    nc.vector.tensor_mul(h_t, so, tc_t)

                    if t == t_steps - 1:
                        return

                    buf = t % 2
                    hTp = psX_pool.tile([128, BC], F32R, tag=f"hTp{c}")
                    nc.tensor.transpose(hTp, h_t, id16)
                    cp = nc.vector.tensor_copy(htx_pend[c], hTp)
                    g_ex[c] += 1
                    if "noex" in ablate:
                        hT_cur[c] = hT_recv[c][buf]
                        return
                    trig = nc.gpsimd.trigger_dma(count=1)
                    add_dep_helper(
                        trig.ins, cp.ins,
                        reason="fire broadcast after h^T staged",
                    )
                    add_dep_helper(
                        trig.ins, prev_gp.ins,
                        reason="trigger order matches prep order",
                    )
                    prev_gp = trig
                    # The arrival wait lives in a critical section: opaque
                    # to Tile's no-exec scheduling sim (which cannot model
                    # remote sem credits).  The 1-elem pad copy reads htx
                    # (pre_crit then waits for the staged h^T) and writes
                    # hT_recv's pad col (post_crit then carries the recv
                    # buffer for the next step's matmuls).
                    with tc.tile_critical(name=f"hx{t}_{c}"):
                        nc.vector.tensor_copy(
                            hT_recv[c][buf][0:1, PADC:PADC + 1],
                            htx_pend[c][0:1, 0:1],
                        )
                        wt = nc.sync.wait_ge(arrv[c], 16 * g_ex[c])
                        add_dep_helper(
                            wt.ins, trig.ins,
                            reason="own trigger before blocking wait",
                        )
                        if g_ex[c] == 1:
                            add_dep_helper(
                                wt.ins, bar.ins,
                                reason="first wait after barrier",
                            )
                    hT_cur[c] = hT_recv[c][buf]

                for t in range(t_steps if "aonly" not in ablate else 0):
                    last = t == t_steps - 1
                    xzt = []
                    for c in range(NCH):
                        xz = xzt_pool.tile([BC, NS], F32R, tag=f"xzt{c}")
                        nc.sync.dma_start(
                            xz, xzb_d.ap()[ds(t * B + BC * c, BC), :]
                        )
                        xzt.append(xz)
                    htx_new = [None, None]
                    if not last:
                        for c in range(NCH):
                            # Descgen early: encodes only the htx address
                            # (slot t%2, last written at t-2); data is read
                            # at trigger time, gated on the htx write.
                            hx = hx_pool.tile([128, BC], F32R, tag=f"htx{c}")
                            htx_new[c] = hx
                            if "noex" not in ablate:
                                prep = nc.gpsimd.remote_dma_broadcast(
                                    hT_recv[c][t % 2][:, ds(pofs, BC)],
                                    hx[:, :],
                                    remote_sem=arrv[c],
                                    local_sem=lsem,
                                    rdests=[(0, m) for m in range(NCORES)],
                                )
                                add_dep_helper(
                                    prep.ins, prev_gp.ins,
                                    reason="SWDGE FIFO: prep after prev trig",
                                )
                                prev_gp = prep

                    if t > 0:
                        emit_tail(1, t - 1)
                        if (t - 1) % RSTEPS == RSTEPS - 1:
                            flush_ring(t - 1)
                    htx_pend[0] = htx_new[0]
                    htx_pend[1] = htx_new[1]

                    # Fill the arrival-wait PE gap: a phase-A tile on even
                    # steps while any remain, else a few warmup transposes
                    # to hold the PE clock at full pstate.
                    m_a = t // 2 + 2
                    if t % 2 == 0 and m_a < ntiles:
                        emit_phase_a_tile(m_a)
                    elif not last:
                        for _w in range(WARM):
                            wp = psT_pool.tile([128, 128], F32, tag="xTp")
                            nc.tensor.transpose(wp, id128, id128)

                    for c in range(NCH):
                        zp = psB_pool.tile([BC, NS], F32, tag=f"zp{c}")
                        zps_pend[c] = zp
                        if t == 0 or "nomm" in ablate:
                            nc.tensor.matmul(zp, id16, xzt[c],
                                             start=True, stop=True)
                        else:
                            nc.tensor.matmul(zp, id16, xzt[c],
                                             start=True, stop=False)
                            for j in range(H // 128):
                                nc.tensor.matmul(
                                    zp, hT_cur[c][:, ts(j, BC)], uk[j],
                                    start=False, stop=(j == H // 128 - 1),
                                )

                    emit_tail(0, t)

                if "aonly" not in ablate:
                    emit_tail(1, t_steps - 1)
                    flush_ring(t_steps - 1)

    nc.compile()
    return nc


def _make_in_maps(x, W, U, b, t_steps: int = T):
    x = np.asarray(x, np.float32)[:, :t_steps, :]
    W = np.asarray(W, np.float32)
    U = np.asarray(U, np.float32)
    b = np.asarray(b, np.float32)
    in_maps = []
    for k in range(NCORES):
        # per-core gate column order: [i | f | o | g]
        cols = np.concatenate(
            [np.arange(k * HS, (k + 1) * HS) + gofs * H for gofs in (0, 1, 3, 2)]
        )
        in_maps.append(
            {
                "x": np.ascontiguousarray(x),
                "w": np.ascontiguousarray(W[:, cols]),
                "u": np.ascontiguousarray(U[:, cols]),
                "b": np.ascontiguousarray(b[cols]).reshape(1, NS),
            }
        )
    return in_maps


def _pjrt_bundle(nc, n_reps: int = 1):
    """Reusable sharded PJRT executable (mirrors bass2jax.run_bass_via_pjrt's
    multi-core branch, but keeps the jitted callable for repeated runs).

    n_reps > 1 chains that many NEFF executions inside one jit call by
    threading the donated output buffer through, so fixed dispatch overhead
    can be cancelled via slope timing."""
    import jax
    from jax.experimental.shard_map import shard_map
    from jax.sharding import Mesh, PartitionSpec
    from concourse import bass2jax

    bass2jax.install_neuronx_cc_hook()
    partition_name = nc.partition_id_tensor.name if nc.partition_id_tensor else None
    in_names, out_names, out_avals, zero_outs = [], [], [], []
    for alloc in nc.m.functions[0].allocations:
        if not isinstance(alloc, mybir.MemoryLocationSet):
            continue
        name = alloc.memorylocations[0].name
        if alloc.kind == "ExternalInput":
            if name != partition_name:
                in_names.append(name)
        elif alloc.kind == "ExternalOutput":
            shape = tuple(alloc.tensor_shape)
            dtype = mybir.dt.np(alloc.dtype)
            out_names.append(name)
            out_avals.append(jax.core.ShapedArray(shape, dtype))
            zero_outs.append(np.zeros(shape, dtype))
    n_params = len(in_names)
    n_outs = len(out_avals)
    all_in_names = list(in_names) + list(out_names)
    if partition_name is not None:
        all_in_names.append(partition_name)

    def _body(*args):
        ins = list(args[:n_params])
        zs = list(args[n_params:])
        for _ in range(n_reps):
            operands = ins + zs
            if partition_name is not None:
                operands.append(bass2jax.partition_id_tensor())
            outs = bass2jax._bass_exec_p.bind(
                *operands,
                out_avals=tuple(out_avals),
                in_names=tuple(all_in_names),
                out_names=tuple(out_names),
                lowering_input_output_aliases=(),
                sim_require_finite=True,
                sim_require_nnan=True,
                nc=nc,
            )
            zs = list(outs)
        return tuple(outs)

    devices = jax.devices()[:NCORES]
    mesh = Mesh(np.asarray(devices), ("core",))
    in_specs = (PartitionSpec("core"),) * (n_params + n_outs)
    out_specs = (PartitionSpec("core"),) * n_outs
    sharded = jax.jit(
        shard_map(
            _body, mesh=mesh, in_specs=in_specs, out_specs=out_specs, check_rep=False
        ),
        donate_argnums=tuple(range(n_params, n_params + n_outs)),
        keep_unused=True,
    )
    return dict(
        fn=sharded,
        mesh=mesh,
        in_names=in_names,
        out_names=out_names,
        out_avals=out_avals,
        zero_outs=zero_outs,
        n_params=n_params,
    )


def _run(inputs, t_steps: int = T, trace: bool = False):
    nc = _build(t_steps)
    in_maps = _make_in_maps(inputs["x"], inputs["W"], inputs["U"], inputs["b"], t_steps)
    res = bass_utils.run_bass_kernel_spmd(
        nc, in_maps, core_ids=list(range(NCORES)), trace=trace
    )
    out = np.empty((B, t_steps, H), np.float32)
    for k in range(NCORES):
        out[:, :, k * HS:(k + 1) * HS] = res.results[k]["hs"]
    return out, res


def kernel(**inputs) -> np.ndarray:
    out, _ = _run(inputs)
    return out
